# revision 11
# baseline (speedup 1.0000x reference)
"""Trainium2 Bass kernel for CausalFlowModel (RNN scan + 2 MLPs + combinator).

Sharding: data-parallel over batch across 8 NeuronCores (64 rows/core).
All weights replicated, pre-transposed+packed on host into lhsT tile banks.
Everything on-device runs in bf16 with fp32 PSUM accumulation; biases are
folded into the matmuls via an appended ones-row.

Layout convention: all activations live TRANSPOSED in SBUF as
[feature-partition, batch-column] so the 511-step recurrence needs no
per-step transposes.

RNN structure (v2): the u-projections are BATCHED out of the step loop.
For each group of G=8 steps, 4 matmuls of N=512 (one per 128-row h block)
write  S*(Wu_aug @ u_aug[:, group])  into a [128, 4, 512] PSUM tile
(start=True), establishing the accumulation base for the whole group.
Each step then only runs the 16 recurrence matmuls (N=64) into its
64-column window (start=False), and two tanh activations:
    tanhA: banks 0..1 -> h blocks 0,1     tanhB: banks 2..3 -> h blocks 2,3
Per-step engine budget: PE ~16x34ns + amortized u-fill; ScalarE 2 ACTs
(~430ns each, chained ~350ns).  The step schedule orders the A-half
matmuls first so tanhA issues as early as possible, and the batched
u-matmuls of the NEXT group are pinned behind step tails as PE filler.
"""

import numpy as np
import ml_dtypes

B, T = 512, 512
SD, CD, H = 256, 64, 512
D1, D2 = 1024, 1024
NCORES = 8
BL = B // NCORES          # 64 batch rows per core
CHUNK = 64                # u steps per DMA chunk
NSTEPS = T - 1            # 511 scan steps
G = 8                     # steps per u-projection group
NG = (NSTEPS + G - 1) // G  # 64 groups; last group has 7 steps

_BF = ml_dtypes.bfloat16

_CACHE = {}


def _bf16(a):
    return np.ascontiguousarray(np.asarray(a, np.float32)).astype(_BF)


def _pack_kxm(W, n_m, n_k, k_off=0):
    """lhsT tile bank [128, n_k*n_m*128]; block j=k*n_m+m is
    W[m*128:(m+1)*128, k_off+k*128 : k_off+(k+1)*128].T"""
    cols = []
    for k in range(n_k):
        for m in range(n_m):
            cols.append(W[m * 128:(m + 1) * 128,
                          k_off + k * 128: k_off + (k + 1) * 128].T)
    return np.concatenate(cols, axis=1)


def _pack_head_bias(W, bvec, n_m, width):
    """[width+1, n_m*128]; block m = [W[m*128:(m+1)*128, :width].T ; b[mblock]]"""
    cols = []
    for m in range(n_m):
        blk = np.concatenate(
            [W[m * 128:(m + 1) * 128, :width].T,
             bvec[m * 128:(m + 1) * 128][None, :]], axis=0)
        cols.append(blk)
    return np.concatenate(cols, axis=1)


def _weight_arrays(inp):
    i2h_W, i2h_b = inp["i2h_W"], inp["i2h_b"]
    w = {
        "whT": _pack_kxm(i2h_W, 4, 4, k_off=CD),
        "wuT": _pack_head_bias(i2h_W, i2h_b, 4, CD),          # [65, 512]
        "x1T": _pack_kxm(inp["x1_W"], 8, 2, k_off=1),
        "x1tb": _pack_head_bias(inp["x1_W"], inp["x1_b"], 8, 1),  # [2, 1024]
        "x2T": _pack_kxm(inp["x2_W"], 8, 8),
        "x2b": np.asarray(inp["x2_b"], np.float32)[None, :],
        "x3T": _pack_kxm(inp["x3_W"], 2, 8),
        "x3b": np.asarray(inp["x3_b"], np.float32)[None, :],
        "u1T": _pack_kxm(inp["u1_W"], 8, 2, k_off=1),
        "u1tb": _pack_head_bias(inp["u1_W"], inp["u1_b"], 8, 1),
        "u2T": _pack_kxm(inp["u2_W"], 8, 8),
        "u2b": np.asarray(inp["u2_b"], np.float32)[None, :],
        "u3T": _pack_kxm(inp["u3_W"], 2, 8),
        "u3b": np.asarray(inp["u3_b"], np.float32)[None, :],
        "h2oT": _pack_kxm(inp["h2o_W"], 2, 4, k_off=CD),
        "h2o_uT": _pack_head_bias(inp["h2o_W"], inp["h2o_b"], 2, CD),  # [65, 256]
        "combT": _pack_kxm(inp["comb_W"], 2, 4),
        "combb": np.asarray(inp["comb_b"], np.float32)[None, :],
    }
    return {k: _bf16(v) for k, v in w.items()}


def _per_core_arrays(inp, c):
    t = np.asarray(inp["t"], np.float32)
    x = np.asarray(inp["x"], np.float32)
    u = np.asarray(inp["u"], np.float32)
    b0 = c * BL
    us = u[:, b0:b0 + BL, :].transpose(2, 0, 1).reshape(CD, T * BL)
    u_aug = np.concatenate([us, np.ones((1, T * BL), np.float32)], axis=0)
    xT = x[b0:b0 + BL].T                              # [256, BL]
    xt = np.concatenate([xT[:128], xT[128:]], axis=1)  # [128, 2*BL]
    tb = np.stack([t[b0:b0 + BL, 0], np.ones(BL, np.float32)], axis=0)  # [2, BL]
    return {"u_aug": _bf16(u_aug), "xt": _bf16(xt), "tb": _bf16(tb)}


def _build_program():
    import concourse.bass as bass
    import concourse.mybir as mybir
    from concourse import bacc
    from concourse.tile import TileContext, add_dep_helper

    bf = mybir.dt.bfloat16
    f32 = mybir.dt.float32
    TANH = mybir.ActivationFunctionType.Tanh

    nc = bacc.Bacc("TRN2", target_bir_lowering=False, debug=False)

    d_in = {}
    def din(name, shape, dt=bf):
        d_in[name] = nc.dram_tensor(name, list(shape), dt, kind="ExternalInput")
        return d_in[name]

    u_aug_d = din("u_aug", (CD + 1, T * BL))
    xt_d = din("xt", (128, 2 * BL))
    tb_d = din("tb", (2, BL))
    wh_d = din("whT", (128, 16 * 128))
    wu_d = din("wuT", (CD + 1, 4 * 128))
    x1_d = din("x1T", (128, 16 * 128))
    x1tb_d = din("x1tb", (2, 8 * 128))
    x2_d = din("x2T", (128, 64 * 128))
    x2b_d = din("x2b", (1, 8 * 128))
    x3_d = din("x3T", (128, 16 * 128))
    x3b_d = din("x3b", (1, 2 * 128))
    u1_d = din("u1T", (128, 16 * 128))
    u1tb_d = din("u1tb", (2, 8 * 128))
    u2_d = din("u2T", (128, 64 * 128))
    u2b_d = din("u2b", (1, 8 * 128))
    u3_d = din("u3T", (128, 16 * 128))
    u3b_d = din("u3b", (1, 2 * 128))
    h2o_d = din("h2oT", (128, 8 * 128))
    h2ou_d = din("h2o_uT", (CD + 1, 2 * 128))
    comb_d = din("combT", (128, 8 * 128))
    combb_d = din("combb", (1, 2 * 128))
    out_d = nc.dram_tensor("out", [2 * 128, BL], f32, kind="ExternalOutput")

    with TileContext(nc) as tc:
        with (
            tc.tile_pool(name="consts", bufs=1) as consts,
            tc.tile_pool(name="upool", bufs=3) as upool,
            tc.tile_pool(name="hpool", bufs=3) as hpool,
            tc.tile_pool(name="work", bufs=1) as work,
        ):
            def cload(dram, shape, dt=bf, name=None):
                tile = consts.tile(list(shape), dt, name=name)
                nc.sync.dma_start(out=tile[:, :], in_=dram[:, :])
                return tile

            # --- DMAs the RNN needs first: head of u-chunk 0 (groups 0-1),
            # wu, wh k-slices, then the rest of chunk 0 ---
            u_tiles = {}
            u_tiles[0] = upool.tile([CD + 1, CHUNK * BL], bf, name="ut")
            head = 2 * G * BL
            nc.sync.dma_start(out=u_tiles[0][:, 0:head],
                              in_=u_aug_d[:, 0:head])
            wu_sb = cload(wu_d, (CD + 1, 4 * 128), name="wu_sb")
            wh_sb = consts.tile([128, 16 * 128], bf, name="wh_sb")
            for kk in range(4):
                nc.sync.dma_start(out=wh_sb[:, kk * 512:(kk + 1) * 512],
                                  in_=wh_d[:, kk * 512:(kk + 1) * 512])
            nc.sync.dma_start(out=u_tiles[0][:, head:CHUNK * BL],
                              in_=u_aug_d[:, head:CHUNK * BL])
            # --- remaining consts (stream in during the RNN) ---
            tb_sb = cload(tb_d, (2, BL), name="tb_sb")
            ones_sb = consts.tile([1, BL], bf, name="ones_sb")
            nc.sync.dma_start(out=ones_sb[:, :], in_=tb_d[1:2, :])
            xt_sb = cload(xt_d, (128, 2 * BL), name="xt_sb")
            h2o_sb = cload(h2o_d, (128, 8 * 128), name="h2o_sb")
            h2ou_sb = cload(h2ou_d, (CD + 1, 2 * 128), name="h2ou_sb")
            x1_sb = cload(x1_d, (128, 16 * 128), name="x1_sb")
            x1tb_sb = cload(x1tb_d, (2, 8 * 128), name="x1tb_sb")
            x2_sb = cload(x2_d, (128, 64 * 128), name="x2_sb")
            x2b_sb = cload(x2b_d, (1, 8 * 128), name="x2b_sb")
            x3_sb = cload(x3_d, (128, 16 * 128), name="x3_sb")
            x3b_sb = cload(x3b_d, (1, 2 * 128), name="x3b_sb")
            u1_sb = cload(u1_d, (128, 16 * 128), name="u1_sb")
            u1tb_sb = cload(u1tb_d, (2, 8 * 128), name="u1tb_sb")
            u2_sb = cload(u2_d, (128, 64 * 128), name="u2_sb")
            u2b_sb = cload(u2b_d, (1, 8 * 128), name="u2b_sb")
            u3_sb = cload(u3_d, (128, 16 * 128), name="u3_sb")
            u3b_sb = cload(u3b_d, (1, 2 * 128), name="u3b_sb")
            comb_sb = cload(comb_d, (128, 8 * 128), name="comb_sb")
            combb_sb = cload(combb_d, (1, 2 * 128), name="combb_sb")

            mm = nc.tensor.matmul
            rnnps_ctx = tc.tile_pool(name="rnnps", bufs=2, space="PSUM")
            rnnps = rnnps_ctx.__enter__()

            # --- PE warm-up + ACT table preload during the initial DMA
            # window: junk matmuls on a memset tile keep the HAM clock gate
            # open and a junk tanh preloads the TANH table set.
            junk_sb = work.tile([128, 640], bf, name="junk_sb")
            nc.vector.memset(junk_sb[:, :], 0.0)
            warm_out = work.tile([128, 64], bf, name="warm_out")
            nc.scalar.activation(warm_out[:, :], junk_sb[:, 0:64], TANH)
            nc.scalar.activation(warm_out[:, :], junk_sb[:, 0:64], TANH)

            # ---------------- RNN scan (symmetric 2+2 bank split) ---------
            # group g covers steps 8g .. min(8g+8, 511)-1.
            # PSUM tiles per group: AB [128, 2, 512] (m 0,1), CD (m 2,3).
            # Step order: Bl(4) Be(4) tanhB Ae(4) Al(4) tanhA [u].
            # Critical chain: tanhA(t) -> Be(t+1) -> tanhB(t+1) -> tanhA(t+1).
            # The batched u-matmul is sync-pinned after the step's tanhB so
            # its 213ns stream never overlaps the Be->tanhB semaphore window.
            ps_tiles = {}

            def group_steps(g):
                return min(NSTEPS - g * G, G)

            def emit_u_group_mm(g, m, after=None, sync=False):
                if m == 0:
                    ps_tiles[g] = (
                        rnnps.tile([128, 2, 512], f32, name="psAB"),
                        rnnps.tile([128, 2, 512], f32, name="psCD"),
                    )
                ns = group_steps(g)
                c = (g * G) // CHUNK
                off = (g * G - c * CHUNK) * BL
                urhs = u_tiles[c][:, off:off + ns * BL]
                tile = ps_tiles[g][m // 2]
                inst = mm(tile[:, m % 2, 0:ns * BL],
                          wu_sb[:, 128 * m:128 * (m + 1)], urhs,
                          start=True, stop=(g == NG - 1), skip_group_check=True)
                if after is not None:
                    add_dep_helper(inst.ins, after.ins, sync=sync,
                                   reason="pin u-fill out of the sem window")

            # warm-up matmuls: ~17 junk N=512 matmuls bridge the PE activity
            # from ~8us (memset done) to the first real u matmul (~11.5us).
            warm_ps = rnnps.tile([128, 2, 512], f32, name="psAB")
            for _ in range(17):
                mm(warm_ps[:, 0, :], junk_sb[:, 0:128], junk_sb[:, 128:640],
                   start=True, stop=True, skip_group_check=True)

            for m in range(4):
                emit_u_group_mm(0, m)

            hcur = None   # (hA tile [128,2,BL], hB tile [128,2,BL])
            for t in range(NSTEPS):
                g, j = divmod(t, G)
                psAB, psCD = ps_tiles[g]
                W = slice(j * BL, (j + 1) * BL)
                hA = hpool.tile([128, 2, BL], bf, name="hA")
                hB = hpool.tile([128, 2, BL], bf, name="hB")
                tanhB_inst = None
                if t > 0:
                    pA, pB = hcur

                    def hmm(m, k, stop):
                        tile = psAB if m < 2 else psCD
                        rhs = (pA if k < 2 else pB)[:, k % 2, :]
                        return mm(tile[:, m % 2, W],
                                  wh_sb[:, 128 * (k * 4 + m):
                                        128 * (k * 4 + m + 1)],
                                  rhs, start=False, stop=stop,
                                  skip_group_check=True)
                    for m_, k_ in ((2, 2), (2, 3), (3, 2), (3, 3)):
                        hmm(m_, k_, False)
                    for m_, k_ in ((2, 0), (3, 0), (2, 1), (3, 1)):
                        hmm(m_, k_, k_ == 1)
                    tanhB_inst = nc.scalar.activation(hB[:, :, :],
                                                      psCD[:, :, W], TANH)
                    for m_, k_ in ((0, 0), (1, 0), (0, 1), (1, 1)):
                        hmm(m_, k_, False)
                    for m_, k_ in ((0, 2), (1, 2), (0, 3), (1, 3)):
                        hmm(m_, k_, k_ == 3)
                    nc.scalar.activation(hA[:, :, :], psAB[:, :, W], TANH)
                else:
                    tanhB_inst = nc.scalar.activation(hB[:, :, :],
                                                      psCD[:, :, W], TANH)
                    nc.scalar.activation(hA[:, :, :], psAB[:, :, W], TANH)

                # next-group batched u-matmuls
                if j in (1, 3, 5, 7) and g + 1 < NG:
                    emit_u_group_mm(g + 1, (j - 1) // 2)
                # u-chunk DMA prefetch, ~4 groups ahead of first use.
                if j == 0 and g + 4 < NG and (g + 4) % (CHUNK // G) == 0:
                    c = (g + 4) // (CHUNK // G)
                    if c <= (T * BL - 1) // (CHUNK * BL):
                        u_tiles[c] = upool.tile([CD + 1, CHUNK * BL], bf,
                                                name="ut")
                        nc.sync.dma_start(
                            out=u_tiles[c][:, :],
                            in_=u_aug_d[:, c * CHUNK * BL:
                                        (c + 1) * CHUNK * BL])
                hcur = (hA, hB)
            rnnps_ctx.__exit__(None, None, None)
            mlpps_ctx = tc.tile_pool(name="mlpps", bufs=4, space="PSUM")
            mlpps = mlpps_ctx.__enter__()

            # ---------------- h2o: r = tanh(h2o_W @ [u_last; h_last] + b) ----
            c_last = (T - 1) // CHUNK
            uc_last = ((T - 1) % CHUNK) * BL
            u_last_tile = u_tiles[c_last]
            ps = mlpps.tile([128, 512], f32, name="mlp_ps")
            for m in range(2):
                mm(ps[:, BL * m:BL * (m + 1)],
                   h2ou_sb[:, 128 * m:128 * (m + 1)],
                   u_last_tile[:, uc_last:uc_last + BL], start=(m == 0),
                   stop=False, skip_group_check=True)
                for k in range(4):
                    j = k * 2 + m
                    hsrc = hcur[0] if k < 2 else hcur[1]
                    mm(ps[:, BL * m:BL * (m + 1)],
                       h2o_sb[:, 128 * j:128 * (j + 1)],
                       hsrc[:, k % 2, :],
                       start=False, stop=(k == 3), skip_group_check=True)
            r_sb = work.tile([128, 2 * BL], bf, name="r_sb")
            nc.scalar.activation(r_sb[:, :], ps[:, 0:2 * BL], TANH)

            # ---------------- MLPs (x and u chains interleaved) ----------
            # Each layer-half is an independent chunk (matmuls + one tanh).
            # Interleaving the two MLPs lets each chain's ACT latency hide
            # under the other chain's matmuls.
            def mlp_chunks(w1_sb, w1tb_sb, w2_sb, w2b_sb, w3_sb, w3b_sb,
                           in_blocks, tag):
                a1 = work.tile([128, 8 * BL], bf, name=tag + "a1")
                a2 = work.tile([128, 8 * BL], bf, name=tag + "a2")
                dst = work.tile([128, 2 * BL], bf, name=tag + "dst")

                def L1_half(half):
                    ps = mlpps.tile([128, 512], f32, name="mlp_ps")
                    for mi in range(4):
                        m = half * 4 + mi
                        o = ps[:, BL * mi:BL * (mi + 1)]
                        mm(o, w1tb_sb[:, 128 * m:128 * (m + 1)], tb_sb[:, :],
                           start=(mi == 0), stop=False, skip_group_check=True)
                        for k in range(2):
                            jj = k * 8 + m
                            mm(o, w1_sb[:, 128 * jj:128 * (jj + 1)],
                               in_blocks[k](), start=False, stop=(k == 1),
                               skip_group_check=True)
                    nc.scalar.activation(
                        a1[:, 4 * BL * half:4 * BL * (half + 1)],
                        ps[:, 0:4 * BL], TANH)

                def L2_half(half):
                    ps = mlpps.tile([128, 512], f32, name="mlp_ps")
                    for mi in range(4):
                        m = half * 4 + mi
                        o = ps[:, BL * mi:BL * (mi + 1)]
                        mm(o, w2b_sb[:, 128 * m:128 * (m + 1)], ones_sb[:, :],
                           start=(mi == 0), stop=False, skip_group_check=True)
                        for k in range(8):
                            jj = k * 8 + m
                            mm(o, w2_sb[:, 128 * jj:128 * (jj + 1)],
                               a1[:, BL * k:BL * (k + 1)],
                               start=False, stop=(k == 7),
                               skip_group_check=True)
                    nc.scalar.activation(
                        a2[:, 4 * BL * half:4 * BL * (half + 1)],
                        ps[:, 0:4 * BL], TANH)

                def L3():
                    ps = mlpps.tile([128, 512], f32, name="mlp_ps")
                    for m in range(2):
                        o = ps[:, BL * m:BL * (m + 1)]
                        mm(o, w3b_sb[:, 128 * m:128 * (m + 1)], ones_sb[:, :],
                           start=(m == 0), stop=False, skip_group_check=True)
                        for k in range(8):
                            jj = k * 2 + m
                            mm(o, w3_sb[:, 128 * jj:128 * (jj + 1)],
                               a2[:, BL * k:BL * (k + 1)],
                               start=False, stop=(k == 7),
                               skip_group_check=True)
                    nc.vector.tensor_copy(dst[:, :], ps[:, 0:2 * BL])

                return dst, [lambda: L1_half(0), lambda: L1_half(1),
                             lambda: L2_half(0), lambda: L2_half(1), L3]

            s_sb, xc = mlp_chunks(x1_sb, x1tb_sb, x2_sb, x2b_sb, x3_sb, x3b_sb,
                                  [lambda: xt_sb[:, 0:BL],
                                   lambda: xt_sb[:, BL:2 * BL]], "x")
            c_sb, uc = mlp_chunks(u1_sb, u1tb_sb, u2_sb, u2b_sb, u3_sb, u3b_sb,
                                  [lambda: r_sb[:, 0:BL],
                                   lambda: r_sb[:, BL:2 * BL]], "u")
            # interleave: x L1 halves first (independent of r), then alternate
            for chunk in (xc[0], xc[1], uc[0], xc[2], uc[1], xc[3],
                          uc[2], xc[4], uc[3], uc[4]):
                chunk()

            # ---------------- combinator ----------------
            ps = mlpps.tile([128, 512], f32, name="mlp_ps")
            for m in range(2):
                o = ps[:, BL * m:BL * (m + 1)]
                mm(o, combb_sb[:, 128 * m:128 * (m + 1)], ones_sb[:, :],
                   start=(m == 0), stop=False, skip_group_check=True)
                for k in range(4):
                    jj = k * 2 + m
                    rhs = (s_sb[:, BL * k:BL * (k + 1)] if k < 2
                           else c_sb[:, BL * (k - 2):BL * (k - 1)])
                    mm(o, comb_sb[:, 128 * jj:128 * (jj + 1)], rhs,
                       start=False, stop=(k == 3), skip_group_check=True)
            out_sb = work.tile([128, 2 * BL], f32, name="out_sb")
            nc.vector.tensor_copy(out_sb[:, :], ps[:, 0:2 * BL])
            nc.sync.dma_start(out=out_d[0:128, :], in_=out_sb[:, 0:BL])
            nc.sync.dma_start(out=out_d[128:256, :], in_=out_sb[:, BL:2 * BL])
            mlpps_ctx.__exit__(None, None, None)

    nc.compile()
    return nc


def _get_program():
    if "nc" not in _CACHE:
        _CACHE["nc"] = _build_program()
    return _CACHE["nc"]


def run(inputs, trace=False, trace_cores=None):
    from concourse.bass_utils import run_bass_kernel_spmd

    nc = _get_program()
    w = _weight_arrays(inputs)
    in_maps = []
    for c in range(NCORES):
        m = dict(w)
        m.update(_per_core_arrays(inputs, c))
        in_maps.append(m)
    res = run_bass_kernel_spmd(nc, in_maps, list(range(NCORES)),
                               trace=trace, trace_cores=trace_cores)
    out = np.empty((B, SD), np.float32)
    for c in range(NCORES):
        out[c * BL:(c + 1) * BL, :] = np.asarray(res.results[c]["out"]).T
    return out, res


def kernel(**inputs):
    out, _ = run(inputs)
    return out


# revision 13
# speedup vs baseline: 1.1990x; 1.1990x over previous
"""Trainium2 Bass kernel for CausalFlowModel (RNN scan + 2 MLPs + combinator).

Sharding: data-parallel over batch across 8 NeuronCores (64 rows/core).
All weights replicated, pre-transposed+packed on host into lhsT tile banks.
Everything on-device runs in bf16 with fp32 PSUM accumulation; biases are
folded into the matmuls via an appended ones-row.

Layout convention: all activations live TRANSPOSED in SBUF as
[feature-partition, batch-column] so the 511-step recurrence needs no
per-step transposes.

RNN structure (v2): the u-projections are BATCHED out of the step loop.
For each group of G=8 steps, 4 matmuls of N=512 (one per 128-row h block)
write  S*(Wu_aug @ u_aug[:, group])  into a [128, 4, 512] PSUM tile
(start=True), establishing the accumulation base for the whole group.
Each step then only runs the 16 recurrence matmuls (N=64) into its
64-column window (start=False), and two tanh activations:
    tanhA: banks 0..1 -> h blocks 0,1     tanhB: banks 2..3 -> h blocks 2,3
Per-step engine budget: PE ~16x34ns + amortized u-fill; ScalarE 2 ACTs
(~430ns each, chained ~350ns).  The step schedule orders the A-half
matmuls first so tanhA issues as early as possible, and the batched
u-matmuls of the NEXT group are pinned behind step tails as PE filler.
"""

import numpy as np
import ml_dtypes

B, T = 512, 512
SD, CD, H = 256, 64, 512
D1, D2 = 1024, 1024
NCORES = 8
BL = B // NCORES          # 64 batch rows per core
CHUNK = 64                # u steps per DMA chunk
NSTEPS = T - 1            # 511 scan steps
G = 8                     # steps per u-projection group
NG = (NSTEPS + G - 1) // G  # 64 groups; last group has 7 steps

_BF = ml_dtypes.bfloat16

_CACHE = {}


def _bf16(a):
    return np.ascontiguousarray(np.asarray(a, np.float32)).astype(_BF)


def _pack_kxm(W, n_m, n_k, k_off=0):
    """lhsT tile bank [128, n_k*n_m*128]; block j=k*n_m+m is
    W[m*128:(m+1)*128, k_off+k*128 : k_off+(k+1)*128].T"""
    cols = []
    for k in range(n_k):
        for m in range(n_m):
            cols.append(W[m * 128:(m + 1) * 128,
                          k_off + k * 128: k_off + (k + 1) * 128].T)
    return np.concatenate(cols, axis=1)


def _pack_head_bias(W, bvec, n_m, width):
    """[width+1, n_m*128]; block m = [W[m*128:(m+1)*128, :width].T ; b[mblock]]"""
    cols = []
    for m in range(n_m):
        blk = np.concatenate(
            [W[m * 128:(m + 1) * 128, :width].T,
             bvec[m * 128:(m + 1) * 128][None, :]], axis=0)
        cols.append(blk)
    return np.concatenate(cols, axis=1)


def _weight_arrays(inp):
    i2h_W, i2h_b = inp["i2h_W"], inp["i2h_b"]
    w = {
        "whT": _pack_kxm(i2h_W, 4, 4, k_off=CD),
        "wuT": _pack_head_bias(i2h_W, i2h_b, 4, CD),          # [65, 512]
        "x1T": _pack_kxm(inp["x1_W"], 8, 2, k_off=1),
        "x1tb": _pack_head_bias(inp["x1_W"], inp["x1_b"], 8, 1),  # [2, 1024]
        "x2T": _pack_kxm(inp["x2_W"], 8, 8),
        "x2b": np.asarray(inp["x2_b"], np.float32)[None, :],
        "x3T": _pack_kxm(inp["x3_W"], 2, 8),
        "x3b": np.asarray(inp["x3_b"], np.float32)[None, :],
        "u1T": _pack_kxm(inp["u1_W"], 8, 2, k_off=1),
        "u1tb": _pack_head_bias(inp["u1_W"], inp["u1_b"], 8, 1),
        "u2T": _pack_kxm(inp["u2_W"], 8, 8),
        "u2b": np.asarray(inp["u2_b"], np.float32)[None, :],
        "u3T": _pack_kxm(inp["u3_W"], 2, 8),
        "u3b": np.asarray(inp["u3_b"], np.float32)[None, :],
        "h2oT": _pack_kxm(inp["h2o_W"], 2, 4, k_off=CD),
        "h2o_uT": _pack_head_bias(inp["h2o_W"], inp["h2o_b"], 2, CD),  # [65, 256]
        "combT": _pack_kxm(inp["comb_W"], 2, 4),
        "combb": np.asarray(inp["comb_b"], np.float32)[None, :],
    }
    return {k: _bf16(v) for k, v in w.items()}


def _per_core_arrays(inp, c):
    t = np.asarray(inp["t"], np.float32)
    x = np.asarray(inp["x"], np.float32)
    u = np.asarray(inp["u"], np.float32)
    b0 = c * BL
    us = u[:, b0:b0 + BL, :].transpose(2, 0, 1).reshape(CD, T * BL)
    u_aug = np.concatenate([us, np.ones((1, T * BL), np.float32)], axis=0)
    xT = x[b0:b0 + BL].T                              # [256, BL]
    xt = np.concatenate([xT[:128], xT[128:]], axis=1)  # [128, 2*BL]
    tb = np.stack([t[b0:b0 + BL, 0], np.ones(BL, np.float32)], axis=0)  # [2, BL]
    return {"u_aug": _bf16(u_aug), "xt": _bf16(xt), "tb": _bf16(tb)}


def _build_program():
    import concourse.bass as bass
    import concourse.mybir as mybir
    from concourse import bacc
    from concourse.tile import TileContext, add_dep_helper

    bf = mybir.dt.bfloat16
    f32 = mybir.dt.float32
    TANH = mybir.ActivationFunctionType.Tanh

    nc = bacc.Bacc("TRN2", target_bir_lowering=False, debug=False)

    d_in = {}
    def din(name, shape, dt=bf):
        d_in[name] = nc.dram_tensor(name, list(shape), dt, kind="ExternalInput")
        return d_in[name]

    u_aug_d = din("u_aug", (CD + 1, T * BL))
    xt_d = din("xt", (128, 2 * BL))
    tb_d = din("tb", (2, BL))
    wh_d = din("whT", (128, 16 * 128))
    wu_d = din("wuT", (CD + 1, 4 * 128))
    x1_d = din("x1T", (128, 16 * 128))
    x1tb_d = din("x1tb", (2, 8 * 128))
    x2_d = din("x2T", (128, 64 * 128))
    x2b_d = din("x2b", (1, 8 * 128))
    x3_d = din("x3T", (128, 16 * 128))
    x3b_d = din("x3b", (1, 2 * 128))
    u1_d = din("u1T", (128, 16 * 128))
    u1tb_d = din("u1tb", (2, 8 * 128))
    u2_d = din("u2T", (128, 64 * 128))
    u2b_d = din("u2b", (1, 8 * 128))
    u3_d = din("u3T", (128, 16 * 128))
    u3b_d = din("u3b", (1, 2 * 128))
    h2o_d = din("h2oT", (128, 8 * 128))
    h2ou_d = din("h2o_uT", (CD + 1, 2 * 128))
    comb_d = din("combT", (128, 8 * 128))
    combb_d = din("combb", (1, 2 * 128))
    out_d = nc.dram_tensor("out", [2 * 128, BL], f32, kind="ExternalOutput")

    with TileContext(nc) as tc:
        with (
            tc.tile_pool(name="consts", bufs=1) as consts,
            tc.tile_pool(name="upool", bufs=3) as upool,
            tc.tile_pool(name="hpool", bufs=3) as hpool,
            tc.tile_pool(name="work", bufs=1) as work,
        ):
            def cload(dram, shape, dt=bf, name=None):
                tile = consts.tile(list(shape), dt, name=name)
                nc.sync.dma_start(out=tile[:, :], in_=dram[:, :])
                return tile

            # --- DMAs the RNN needs first: head of u-chunk 0 (groups 0-1),
            # wu, wh k-slices, then the rest of chunk 0 ---
            u_tiles = {}
            u_tiles[0] = upool.tile([CD + 1, CHUNK * BL], bf, name="ut")
            head = 2 * G * BL
            nc.sync.dma_start(out=u_tiles[0][:, 0:head],
                              in_=u_aug_d[:, 0:head])
            wu_sb = cload(wu_d, (CD + 1, 4 * 128), name="wu_sb")
            wh_sb = consts.tile([128, 16 * 128], bf, name="wh_sb")
            for kk in range(4):
                nc.sync.dma_start(out=wh_sb[:, kk * 512:(kk + 1) * 512],
                                  in_=wh_d[:, kk * 512:(kk + 1) * 512])
            nc.sync.dma_start(out=u_tiles[0][:, head:CHUNK * BL],
                              in_=u_aug_d[:, head:CHUNK * BL])
            # --- remaining consts (stream in during the RNN) ---
            tb_sb = cload(tb_d, (2, BL), name="tb_sb")
            ones_sb = consts.tile([1, BL], bf, name="ones_sb")
            nc.sync.dma_start(out=ones_sb[:, :], in_=tb_d[1:2, :])
            xt_sb = cload(xt_d, (128, 2 * BL), name="xt_sb")
            h2o_sb = cload(h2o_d, (128, 8 * 128), name="h2o_sb")
            h2ou_sb = cload(h2ou_d, (CD + 1, 2 * 128), name="h2ou_sb")
            x1_sb = cload(x1_d, (128, 16 * 128), name="x1_sb")
            x1tb_sb = cload(x1tb_d, (2, 8 * 128), name="x1tb_sb")
            x2_sb = cload(x2_d, (128, 64 * 128), name="x2_sb")
            x2b_sb = cload(x2b_d, (1, 8 * 128), name="x2b_sb")
            x3_sb = cload(x3_d, (128, 16 * 128), name="x3_sb")
            x3b_sb = cload(x3b_d, (1, 2 * 128), name="x3b_sb")
            u1_sb = cload(u1_d, (128, 16 * 128), name="u1_sb")
            u1tb_sb = cload(u1tb_d, (2, 8 * 128), name="u1tb_sb")
            u2_sb = cload(u2_d, (128, 64 * 128), name="u2_sb")
            u2b_sb = cload(u2b_d, (1, 8 * 128), name="u2b_sb")
            u3_sb = cload(u3_d, (128, 16 * 128), name="u3_sb")
            u3b_sb = cload(u3b_d, (1, 2 * 128), name="u3b_sb")
            comb_sb = cload(comb_d, (128, 8 * 128), name="comb_sb")
            combb_sb = cload(combb_d, (1, 2 * 128), name="combb_sb")

            mm = nc.tensor.matmul
            rnnps_ctx = tc.tile_pool(name="rnnps", bufs=2, space="PSUM")
            rnnps = rnnps_ctx.__enter__()

            # --- PE warm-up + ACT table preload during the initial DMA
            # window: junk matmuls on a memset tile keep the HAM clock gate
            # open and a junk tanh preloads the TANH table set.
            junk_sb = work.tile([128, 640], bf, name="junk_sb")
            nc.vector.memset(junk_sb[:, :], 0.0)
            warm_out = work.tile([128, 64], bf, name="warm_out")
            nc.scalar.activation(warm_out[:, :], junk_sb[:, 0:64], TANH)
            nc.scalar.activation(warm_out[:, :], junk_sb[:, 0:64], TANH)

            # ---------------- RNN scan (symmetric 2+2 bank split) ---------
            # group g covers steps 8g .. min(8g+8, 511)-1.
            # PSUM tiles per group: AB [128, 2, 512] (m 0,1), CD (m 2,3).
            # Step order: Bl(4) Be(4) tanhB Ae(4) Al(4) tanhA [u].
            # Critical chain: tanhA(t) -> Be(t+1) -> tanhB(t+1) -> tanhA(t+1).
            # The batched u-matmul is sync-pinned after the step's tanhB so
            # its 213ns stream never overlaps the Be->tanhB semaphore window.
            ps_tiles = {}

            def group_steps(g):
                return min(NSTEPS - g * G, G)

            def emit_u_group_mm(g, m, after=None, sync=False):
                if m == 0:
                    ps_tiles[g] = (
                        rnnps.tile([128, 2, 512], f32, name="psAB"),
                        rnnps.tile([128, 2, 512], f32, name="psCD"),
                    )
                ns = group_steps(g)
                c = (g * G) // CHUNK
                off = (g * G - c * CHUNK) * BL
                urhs = u_tiles[c][:, off:off + ns * BL]
                tile = ps_tiles[g][m // 2]
                inst = mm(tile[:, m % 2, 0:ns * BL],
                          wu_sb[:, 128 * m:128 * (m + 1)], urhs,
                          start=True, stop=(g == NG - 1), skip_group_check=True)
                if after is not None:
                    add_dep_helper(inst.ins, after.ins, sync=sync,
                                   reason="pin u-fill out of the sem window")

            # warm-up matmuls: ~17 junk N=512 matmuls bridge the PE activity
            # from ~8us (memset done) to the first real u matmul (~11.5us).
            warm_ps = rnnps.tile([128, 2, 512], f32, name="psAB")
            for _ in range(17):
                mm(warm_ps[:, 0, :], junk_sb[:, 0:128], junk_sb[:, 128:640],
                   start=True, stop=True, skip_group_check=True)

            for m in range(4):
                emit_u_group_mm(0, m)

            hcur = None   # (hA tile [128,2,BL], hB tile [128,2,BL])
            for t in range(NSTEPS):
                g, j = divmod(t, G)
                psAB, psCD = ps_tiles[g]
                W = slice(j * BL, (j + 1) * BL)
                hA = hpool.tile([128, 2, BL], bf, name="hA")
                hB = hpool.tile([128, 2, BL], bf, name="hB")
                tanhB_inst = None
                if t > 0:
                    pA, pB = hcur

                    def hmm(m, k, stop):
                        tile = psAB if m < 2 else psCD
                        rhs = (pA if k < 2 else pB)[:, k % 2, :]
                        return mm(tile[:, m % 2, W],
                                  wh_sb[:, 128 * (k * 4 + m):
                                        128 * (k * 4 + m + 1)],
                                  rhs, start=False, stop=stop,
                                  skip_group_check=True)
                    for m_, k_ in ((2, 2), (2, 3), (3, 2), (3, 3)):
                        hmm(m_, k_, False)
                    for m_, k_ in ((2, 0), (3, 0), (2, 1), (3, 1)):
                        hmm(m_, k_, k_ == 1)
                    tanhB_inst = nc.scalar.activation(hB[:, :, :],
                                                      psCD[:, :, W], TANH)
                    for m_, k_ in ((0, 0), (1, 0), (0, 1), (1, 1)):
                        hmm(m_, k_, False)
                    for m_, k_ in ((0, 2), (1, 2), (0, 3), (1, 3)):
                        hmm(m_, k_, k_ == 3)
                    nc.scalar.activation(hA[:, :, :], psAB[:, :, W], TANH)
                else:
                    tanhB_inst = nc.scalar.activation(hB[:, :, :],
                                                      psCD[:, :, W], TANH)
                    nc.scalar.activation(hA[:, :, :], psAB[:, :, W], TANH)

                # next-group batched u-matmuls
                if j in (1, 3, 5, 7) and g + 1 < NG:
                    emit_u_group_mm(g + 1, (j - 1) // 2)
                # u-chunk DMA prefetch, ~4 groups ahead of first use.
                if j == 0 and g + 4 < NG and (g + 4) % (CHUNK // G) == 0:
                    c = (g + 4) // (CHUNK // G)
                    if c <= (T * BL - 1) // (CHUNK * BL):
                        u_tiles[c] = upool.tile([CD + 1, CHUNK * BL], bf,
                                                name="ut")
                        nc.sync.dma_start(
                            out=u_tiles[c][:, :],
                            in_=u_aug_d[:, c * CHUNK * BL:
                                        (c + 1) * CHUNK * BL])
                hcur = (hA, hB)
            rnnps_ctx.__exit__(None, None, None)
            mlpps_ctx = tc.tile_pool(name="mlpps", bufs=4, space="PSUM")
            mlpps = mlpps_ctx.__enter__()

            # ---------------- h2o: r = tanh(h2o_W @ [u_last; h_last] + b) ----
            c_last = (T - 1) // CHUNK
            uc_last = ((T - 1) % CHUNK) * BL
            u_last_tile = u_tiles[c_last]
            ps = mlpps.tile([128, 512], f32, name="mlp_ps")
            for m in range(2):
                mm(ps[:, BL * m:BL * (m + 1)],
                   h2ou_sb[:, 128 * m:128 * (m + 1)],
                   u_last_tile[:, uc_last:uc_last + BL], start=(m == 0),
                   stop=False, skip_group_check=True)
                for k in range(4):
                    j = k * 2 + m
                    hsrc = hcur[0] if k < 2 else hcur[1]
                    mm(ps[:, BL * m:BL * (m + 1)],
                       h2o_sb[:, 128 * j:128 * (j + 1)],
                       hsrc[:, k % 2, :],
                       start=False, stop=(k == 3), skip_group_check=True)
            r_sb = work.tile([128, 2 * BL], bf, name="r_sb")
            nc.scalar.activation(r_sb[:, :], ps[:, 0:2 * BL], TANH)

            # ---------------- MLPs (x and u chains interleaved) ----------
            # Each layer-half is an independent chunk (matmuls + one tanh).
            # Interleaving the two MLPs lets each chain's ACT latency hide
            # under the other chain's matmuls.
            def mlp_chunks(w1_sb, w1tb_sb, w2_sb, w2b_sb, w3_sb, w3b_sb,
                           in_blocks, tag):
                a1 = work.tile([128, 8 * BL], bf, name=tag + "a1")
                a2 = work.tile([128, 8 * BL], bf, name=tag + "a2")
                dst = work.tile([128, 2 * BL], bf, name=tag + "dst")

                def L1_half(half):
                    ps = mlpps.tile([128, 512], f32, name="mlp_ps")
                    for mi in range(4):
                        m = half * 4 + mi
                        o = ps[:, BL * mi:BL * (mi + 1)]
                        mm(o, w1tb_sb[:, 128 * m:128 * (m + 1)], tb_sb[:, :],
                           start=(mi == 0), stop=False, skip_group_check=True)
                        for k in range(2):
                            jj = k * 8 + m
                            mm(o, w1_sb[:, 128 * jj:128 * (jj + 1)],
                               in_blocks[k](), start=False, stop=(k == 1),
                               skip_group_check=True)
                    nc.scalar.activation(
                        a1[:, 4 * BL * half:4 * BL * (half + 1)],
                        ps[:, 0:4 * BL], TANH)

                def L2_half(half):
                    ps = mlpps.tile([128, 512], f32, name="mlp_ps")
                    for mi in range(4):
                        m = half * 4 + mi
                        o = ps[:, BL * mi:BL * (mi + 1)]
                        mm(o, w2b_sb[:, 128 * m:128 * (m + 1)], ones_sb[:, :],
                           start=(mi == 0), stop=False, skip_group_check=True)
                        for k in range(8):
                            jj = k * 8 + m
                            mm(o, w2_sb[:, 128 * jj:128 * (jj + 1)],
                               a1[:, BL * k:BL * (k + 1)],
                               start=False, stop=(k == 7),
                               skip_group_check=True)
                    nc.scalar.activation(
                        a2[:, 4 * BL * half:4 * BL * (half + 1)],
                        ps[:, 0:4 * BL], TANH)

                def L3():
                    ps = mlpps.tile([128, 512], f32, name="mlp_ps")
                    for m in range(2):
                        o = ps[:, BL * m:BL * (m + 1)]
                        mm(o, w3b_sb[:, 128 * m:128 * (m + 1)], ones_sb[:, :],
                           start=(m == 0), stop=False, skip_group_check=True)
                        for k in range(8):
                            jj = k * 2 + m
                            mm(o, w3_sb[:, 128 * jj:128 * (jj + 1)],
                               a2[:, BL * k:BL * (k + 1)],
                               start=False, stop=(k == 7),
                               skip_group_check=True)
                    nc.vector.tensor_copy(dst[:, :], ps[:, 0:2 * BL])

                return dst, [lambda: L1_half(0), lambda: L1_half(1),
                             lambda: L2_half(0), lambda: L2_half(1), L3]

            s_sb, xc = mlp_chunks(x1_sb, x1tb_sb, x2_sb, x2b_sb, x3_sb, x3b_sb,
                                  [lambda: xt_sb[:, 0:BL],
                                   lambda: xt_sb[:, BL:2 * BL]], "x")
            c_sb, uc = mlp_chunks(u1_sb, u1tb_sb, u2_sb, u2b_sb, u3_sb, u3b_sb,
                                  [lambda: r_sb[:, 0:BL],
                                   lambda: r_sb[:, BL:2 * BL]], "u")
            # interleave: x L1 halves first (independent of r), then alternate
            for chunk in (xc[0], xc[1], uc[0], xc[2], uc[1], xc[3],
                          uc[2], xc[4], uc[3], uc[4]):
                chunk()

            # ---------------- combinator ----------------
            ps = mlpps.tile([128, 512], f32, name="mlp_ps")
            for m in range(2):
                o = ps[:, BL * m:BL * (m + 1)]
                mm(o, combb_sb[:, 128 * m:128 * (m + 1)], ones_sb[:, :],
                   start=(m == 0), stop=False, skip_group_check=True)
                for k in range(4):
                    jj = k * 2 + m
                    rhs = (s_sb[:, BL * k:BL * (k + 1)] if k < 2
                           else c_sb[:, BL * (k - 2):BL * (k - 1)])
                    mm(o, comb_sb[:, 128 * jj:128 * (jj + 1)], rhs,
                       start=False, stop=(k == 3), skip_group_check=True)
            out_sb = work.tile([128, 2 * BL], f32, name="out_sb")
            nc.vector.tensor_copy(out_sb[:, :], ps[:, 0:2 * BL])
            nc.sync.dma_start(out=out_d[0:128, :], in_=out_sb[:, 0:BL])
            nc.sync.dma_start(out=out_d[128:256, :], in_=out_sb[:, BL:2 * BL])
            mlpps_ctx.__exit__(None, None, None)

    nc.compile()
    return nc


def _get_program():
    if "nc" not in _CACHE:
        _CACHE["nc"] = _build_program()
    return _CACHE["nc"]


def run(inputs, trace=False, trace_cores=None):
    from concourse.bass_utils import run_bass_kernel_spmd

    nc = _get_program()
    w = _weight_arrays(inputs)
    in_maps = []
    for c in range(NCORES):
        m = dict(w)
        m.update(_per_core_arrays(inputs, c))
        in_maps.append(m)
    res = run_bass_kernel_spmd(nc, in_maps, list(range(NCORES)),
                               trace=trace, trace_cores=trace_cores)
    out = np.empty((B, SD), np.float32)
    for c in range(NCORES):
        out[c * BL:(c + 1) * BL, :] = np.asarray(res.results[c]["out"]).T
    return out, res


def kernel(**inputs):
    out, _ = run(inputs)
    return out
